# revision 4
# baseline (speedup 1.0000x reference)
"""Trainium2 Bass kernel for a 4-layer GCN (nn_GCNModel), SPMD across 8 NeuronCores.

Strategy (graph/data parallel per the sharding hint):
  - Nodes are partitioned across the 8 cores (6250 real rows/core, padded to
    6272 = 49 blocks of 128).  Each core owns the edges whose DESTINATION
    falls in its shard, pre-sorted by destination block on the host and
    padded so every core sees the same per-block chunk structure (SPMD: one
    NEFF, 8 cores).
  - GCN normalization (deg -> dis -> per-edge norm = dis[row]*w*dis[col]) is
    computed on the HOST; the kernel receives per-edge norms (bf16) and the
    per-node self-loop coefficients 1/deg directly.
  - Per layer: each core computes h = X_shard @ W for its rows (bf16), the
    padded h table is AllGather'd to every core's DRAM, then each core
    gathers the source rows of its edges with `dma_gather` (one gather per
    (dest-block, lo/hi table half); int16 indices force the lo/hi split at
    32768).  Each gather's index stream is padded to the max real count over
    cores with index 0 and then with trailing -1s, and num_idxs_reg is set
    to that count, so the SWDGE generator skips the chunk-rounding padding
    entirely (the matmul masks stale lanes via norm=0).
  - Norm-scaled one-hot selection matrices for a whole 4-block span are
    built with two broadcast tensor_tensor ops on the Vector engine
    (M[e,c,d] = (iota[d]==rl[e,c]) * norm[e,c], bf16), and aggregation runs
    as TensorE matmul accumulation in PSUM: agg^T[f,d] += G[e,f]^T @ M[e,d].
  - Self-loop edges never enter the gather stream: their contribution
    h[d]*1/deg[d] is added per dest block as one extra accumulation matmul
    lhsT=h_block (node-major, local) rhs=diag((iota==lane)*dinv).
  - agg^T comes out feature-major [F, nodes] — the stationary operand layout
    the next layer's matmul wants; bias+ReLU ride the PSUM->SBUF copy on the
    Scalar engine.
  - All per-core constants ship as ONE packed f32 blob (single DMA/sem).

Self-contained: hardcodes all shapes; host side only computes norms,
reorders/pads indices and concatenates shard outputs.
"""

import os
import sys

import numpy as np
import ml_dtypes

for _p in ("/opt/trn_rl_repo", "/root/.axon_site/_ro/trn_rl_repo"):
    if os.path.isdir(_p) and _p not in sys.path:
        sys.path.append(_p)

from concourse import bacc, mybir, tile  # noqa: E402
from concourse.bass_utils import run_bass_kernel_spmd  # noqa: E402

F32 = mybir.dt.float32
BF16 = mybir.dt.bfloat16
I16 = mybir.dt.int16
BF = ml_dtypes.bfloat16

N_NODES = 50000
NCORES = 8
S_REAL = N_NODES // NCORES           # 6250 real nodes per core
NB = (S_REAL + 127) // 128           # 49 dest blocks per core
S = NB * 128                         # 6272 padded shard size
NPAD = NCORES * S                    # 50176 padded global nodes
SPLIT = 32768                        # int16 index limit for dma_gather
SPAN = 4                             # dest blocks per span (M build / g tile)
WD = 128                             # h-table row width (bf16 -> 256B rows)
MAX_IDX_PER_GATHER = 8192
# (F_in, F_out) per layer
LAYERS = [(128, 100), (100, 100), (100, 50), (50, 6)]

TRACE = False
TRACE_KW = {}
LAST_RESULTS = None


def _wrap_idx16(flat):
    """dma_gather index layout: idx i at (partition i%16, column i//16),
    replicated to all 8 16-partition groups."""
    n = len(flat)
    assert n % 16 == 0
    w = flat.reshape(n // 16, 16).T.astype(np.int16)     # [16, n/16]
    return np.tile(w, (8, 1))                            # [128, n/16]


def _preprocess(edge_index, edge_weight):
    """Shard + sort non-self edges by (dest block, lo/hi col half).

    Returns (spans, block_cols, CH_TOT, idx16, rl_a, nrm_a, dinv_a):
      spans: list of (j0, blocks, groups); groups = (b, h, jc, nch, maxcnt)
      block_cols: per block, the ordered chunk-column indices
    """
    row = edge_index[0].astype(np.int64)
    col = edge_index[1].astype(np.int64)
    w = edge_weight.astype(np.float32)

    # GCN renormalization coefficients on host (self-loop weight 1 included
    # in deg, but self-loop edges are applied separately in the kernel).
    deg = np.ones(N_NODES, np.float32)
    np.add.at(deg, row, w)
    dis = (1.0 / np.sqrt(deg)).astype(np.float32)
    norm = dis[row] * w * dis[col]
    dinv_full = (1.0 / deg).astype(np.float32)

    core = row // S_REAL
    r_in = row % S_REAL
    blk = r_in // 128
    rloc = (r_in % 128).astype(np.float32)
    colp = ((col // S_REAL) * S + col % S_REAL).astype(np.int64)
    hi = (colp >= SPLIT).astype(np.int64)

    counts = np.zeros((NCORES, NB, 2), np.int64)
    np.add.at(counts, (core, blk, hi), 1)
    maxcnt = counts.max(axis=0)                          # [NB, 2]
    CHP = -(-maxcnt // 128)                              # [NB, 2] chunks
    CH_TOT = int(CHP.sum())

    # span/chunk-column layout
    spans = []
    block_cols = [[] for _ in range(NB)]
    part_col = {}
    j = 0
    for s0 in range(0, NB, SPAN):
        blocks = list(range(s0, min(s0 + SPAN, NB)))
        j0 = j
        n_lo = n_hi = 0
        for h in range(2):
            for b in blocks:
                nch = int(CHP[b, h])
                part_col[(b, h)] = j
                block_cols[b].extend(range(j, j + nch))
                j += nch
                if h == 0:
                    n_lo += nch
                else:
                    n_hi += nch
        assert n_lo * 128 <= MAX_IDX_PER_GATHER
        assert n_hi * 128 <= MAX_IDX_PER_GATHER
        spans.append((j0, n_lo, n_hi, blocks))
    assert j == CH_TOT

    idx16 = np.zeros((NCORES, 128, CH_TOT * 8), np.int16)
    rl_a = np.zeros((NCORES, 128, CH_TOT), np.float32)
    nrm_a = np.zeros((NCORES, 128, CH_TOT), np.float32)

    order = np.lexsort((hi, blk, core))
    scolp, srloc, snrm = colp[order], rloc[order], norm[order]
    bounds = np.searchsorted(
        core[order] * (NB * 2) + blk[order] * 2 + hi[order],
        np.arange(NCORES * NB * 2 + 1))

    for c in range(NCORES):
        for b in range(NB):
            for h in range(2):
                nch = int(CHP[b, h])
                if nch == 0:
                    continue
                jc = part_col[(b, h)]
                k = (c * NB + b) * 2 + h
                s0, s1 = bounds[k], bounds[k + 1]
                cnt = s1 - s0
                n = nch * 128
                ii = np.zeros(n, np.int64)
                ii[:cnt] = scolp[s0:s1] - h * SPLIT
                rr = np.zeros(n, np.float32)
                rr[:cnt] = srloc[s0:s1]
                ww = np.zeros(n, np.float32)
                ww[:cnt] = snrm[s0:s1]
                rl_a[c, :, jc:jc + nch] = rr.reshape(nch, 128).T
                nrm_a[c, :, jc:jc + nch] = ww.reshape(nch, 128).T
                idx16[c, :, jc * 8:jc * 8 + n // 16] = _wrap_idx16(ii)

    dinv_a = np.zeros((NCORES, 128, NB), np.float32)
    for c in range(NCORES):
        loc = np.arange(S)
        n_glob = c * S_REAL + loc
        valid = loc < S_REAL
        da = np.where(valid, dinv_full[np.minimum(n_glob, N_NODES - 1)], 0.0)
        dinv_a[c] = da.reshape(NB, 128).T
    return spans, block_cols, CH_TOT, idx16, rl_a, nrm_a, dinv_a


def _blob_offsets(CH_TOT):
    """Column layout of the packed per-core constant blob [128, NCOLS] f32."""
    CH2 = (CH_TOT + 1) // 2 * 2
    off = {}
    o = 0
    for k, n in (("idx16", CH_TOT * 4), ("rl16", CH2 // 2),
                 ("nrm16", CH2 // 2), ("iota", 64), ("xT", S // 2),
                 ("dinv", NB), ("ic", 1)):
        off[k] = o
        o += n
    for l, (fin, fout) in enumerate(LAYERS):
        off[f"W{l}"] = o
        o += fout // 2
    for l, (fin, fout) in enumerate(LAYERS):
        off[f"b{l}"] = o
        o += 1
    return off, o


def _build(spans, block_cols, CH_TOT):
    nc = bacc.Bacc(None, num_devices=NCORES)
    AGRP = [list(range(NCORES))]
    AF = mybir.ActivationFunctionType
    OP = mybir.AluOpType

    off, ncols = _blob_offsets(CH_TOT)
    blob_p = nc.declare_dram_parameter("blob", [128, ncols], F32, isOutput=False)
    out_p = nc.declare_dram_parameter("out", [LAYERS[-1][1], S], F32, isOutput=True)

    with tile.TileContext(nc, num_cores=NCORES) as tc:
        with (
            tc.tile_pool(name="const", bufs=1) as cpool,
            tc.tile_pool(name="xpool", bufs=2) as xpool,
            tc.tile_pool(name="dram", bufs=1, space="DRAM") as dpool,
            tc.tile_pool(name="psum", bufs=2, space="PSUM") as ppool,
            tc.tile_pool(name="work", bufs=3) as wpool,
        ):
            blob = cpool.tile([128, ncols], F32)
            nc.sync.dma_start(out=blob[:], in_=blob_p[:])

            idx16_sb = blob[:, off["idx16"]:off["idx16"] + CH_TOT * 4].bitcast(I16)
            rl16 = blob[:, off["rl16"]:off["nrm16"]].bitcast(BF16)
            nrm16 = blob[:, off["nrm16"]:off["iota"]].bitcast(BF16)
            iota_b = blob[:, off["iota"]:off["iota"] + 64].bitcast(BF16)
            xT0 = blob[:, off["xT"]:off["xT"] + S // 2].bitcast(BF16)
            dinv_sb = blob[:, off["dinv"]:off["dinv"] + NB]
            ic_sb = blob[:, off["ic"]:off["ic"] + 1]
            W_sb = [blob[:LAYERS[l][0],
                         off[f"W{l}"]:off[f"W{l}"] + LAYERS[l][1] // 2].bitcast(BF16)
                    for l in range(4)]
            b_sb = [blob[:LAYERS[l][1], off[f"b{l}"]:off[f"b{l}"] + 1]
                    for l in range(4)]

            # ---- layers ----
            xT = xT0
            for l, (fin, fout) in enumerate(LAYERS):
                # h = X_shard @ W (node-major bf16, padded to WD), one DMA out
                h_all = wpool.tile([128, NB * WD], BF16, tag="hall", bufs=1)
                for b in range(NB):
                    ps_h = ppool.tile([128, fout], F32, tag="ps_small")
                    nc.tensor.matmul(
                        out=ps_h[:], lhsT=xT[:fin, b * 128:(b + 1) * 128],
                        rhs=W_sb[l], start=True, stop=True)
                    nc.scalar.activation(
                        h_all[:, b * WD:b * WD + fout], ps_h[:], AF.Copy)
                hsh = dpool.tile([S, WD], BF16, name=f"hsh{l}")
                nc.sync.dma_start(
                    out=hsh.rearrange("(b p) w -> p b w", p=128),
                    in_=h_all[:].rearrange("p (b w) -> p b w", w=WD))

                htab = dpool.tile([NPAD, WD], BF16, addr_space="Shared",
                                  name=f"htab{l}")
                nc.gpsimd.collective_compute(
                    "AllGather", OP.bypass, replica_groups=AGRP,
                    ins=[hsh[:]], outs=[htab[:]])

                # aggregation: agg^T[f, d] += G[e, f]^T @ M[e, d]
                xT_next = xpool.tile([fout, S], BF16 if l + 1 < len(LAYERS)
                                     else F32, tag="xT", name=f"xT{l + 1}_sb")
                for (j0, n_lo, n_hi, blocks) in spans:
                    nch_s = n_lo + n_hi
                    g = wpool.tile([128, nch_s * WD], BF16, tag="g", bufs=3)
                    for h, tab, o0, nch in ((0, htab[0:SPLIT, :], 0, n_lo),
                                            (1, htab[SPLIT:NPAD, :],
                                             n_lo * WD, n_hi)):
                        if nch == 0:
                            continue
                        i16 = (j0 + (0 if h == 0 else n_lo)) * 8
                        nc.gpsimd.dma_gather(
                            g[:, o0:o0 + nch * WD].rearrange(
                                "p (c w) -> p c w", w=WD),
                            tab, idx16_sb[:, i16:i16 + nch * 8],
                            nch * 128, nch * 128, WD, single_packet=False)
                    # norm-scaled one-hots for the whole span (2 DVE ops)
                    mspan = wpool.tile([128, nch_s * 128], BF16, tag="m", bufs=2)
                    m3 = mspan[:].rearrange("p (c w) -> p c w", w=128)
                    nc.vector.tensor_tensor(
                        out=m3,
                        in0=iota_b.unsqueeze(1).broadcast_to((128, nch_s, 128)),
                        in1=rl16[:, j0:j0 + nch_s].unsqueeze(2).broadcast_to(
                            (128, nch_s, 128)),
                        op=OP.is_equal)
                    nc.vector.tensor_tensor(
                        out=m3, in0=m3,
                        in1=nrm16[:, j0:j0 + nch_s].unsqueeze(2).broadcast_to(
                            (128, nch_s, 128)),
                        op=OP.mult)
                    for b in blocks:
                        cols = block_cols[b]
                        ps_a = ppool.tile([fout, 128], F32, tag="psa", bufs=4)
                        for k, j in enumerate(cols):
                            o0 = (j - j0) * WD
                            nc.tensor.matmul(
                                out=ps_a[:], lhsT=g[:, o0:o0 + fout],
                                rhs=mspan[:, (j - j0) * 128:(j - j0 + 1) * 128],
                                start=(k == 0), stop=False)
                        # self-loop: agg^T[:, d] += h[d, :]^T * dinv[d]
                        diag = wpool.tile([128, 128], BF16, tag="diag", bufs=3)
                        nc.vector.tensor_scalar(
                            diag[:], iota_b, ic_sb[:, 0:1], dinv_sb[:, b:b + 1],
                            OP.is_equal, OP.mult)
                        nc.tensor.matmul(
                            out=ps_a[:], lhsT=h_all[:, b * WD:b * WD + fout],
                            rhs=diag[:], start=(len(cols) == 0), stop=True)
                        # epilogue: +bias, ReLU on the PSUM->SBUF copy
                        nc.scalar.activation(
                            xT_next[:, b * 128:(b + 1) * 128], ps_a[:],
                            AF.Relu if l + 1 < len(LAYERS) else AF.Identity,
                            bias=b_sb[l][:, 0:1])
                xT = xT_next

            nc.sync.dma_start(out=out_p[:], in_=xT[:])
    nc.finalize()   # Bacc: reg alloc + event-sem wait splitting
    return nc


def kernel(x, edge_index, edge_weight, W0, b0, W1, b1, W2, b2, W3, b3):
    global LAST_RESULTS
    x = np.ascontiguousarray(np.asarray(x, np.float32))
    spans, block_cols, CH_TOT, idx16, rl_a, nrm_a, dinv_a = _preprocess(
        np.asarray(edge_index), np.asarray(edge_weight))

    nc = _build(spans, block_cols, CH_TOT)

    off, ncols = _blob_offsets(CH_TOT)
    CH2 = (CH_TOT + 1) // 2 * 2
    Ws = [np.asarray(W, np.float32).astype(BF) for W in (W0, W1, W2, W3)]
    bs = [np.asarray(b, np.float32).reshape(-1) for b in (b0, b1, b2, b3)]

    in_maps = []
    for c in range(NCORES):
        blob = np.zeros((128, ncols), np.float32)
        blob[:, off["idx16"]:off["idx16"] + CH_TOT * 4] = idx16[c].view(np.float32)
        rl16 = np.zeros((128, CH2), BF)
        rl16[:, :CH_TOT] = rl_a[c].astype(BF)
        blob[:, off["rl16"]:off["nrm16"]] = rl16.view(np.float32)
        nrm16 = np.zeros((128, CH2), BF)
        nrm16[:, :CH_TOT] = nrm_a[c].astype(BF)
        blob[:, off["nrm16"]:off["iota"]] = nrm16.view(np.float32)
        blob[:, off["iota"]:off["iota"] + 64] = np.broadcast_to(
            np.arange(128, dtype=np.float32).astype(BF).view(np.float32)[None, :],
            (128, 64))
        xb = np.zeros((128, S), BF)
        xb[:, :S_REAL] = x[c * S_REAL:(c + 1) * S_REAL].T.astype(BF)
        blob[:, off["xT"]:off["xT"] + S // 2] = xb.view(np.float32)
        blob[:, off["dinv"]:off["dinv"] + NB] = dinv_a[c]
        blob[:, off["ic"]] = np.arange(128, dtype=np.float32)
        for l, (fin, fout) in enumerate(LAYERS):
            blob[:fin, off[f"W{l}"]:off[f"W{l}"] + fout // 2] = Ws[l].view(np.float32)
            blob[:fout, off[f"b{l}"]] = bs[l]
        in_maps.append({"blob": blob})

    res = run_bass_kernel_spmd(nc, in_maps, core_ids=list(range(NCORES)),
                               trace=TRACE, trace_kwargs=dict(TRACE_KW))
    LAST_RESULTS = res
    out = np.concatenate(
        [res.results[c]["out"][:, :S_REAL].T for c in range(NCORES)], axis=0)
    return np.ascontiguousarray(out.astype(np.float32))


# revision 5
# speedup vs baseline: 1.2266x; 1.2266x over previous
"""Trainium2 Bass kernel for a 4-layer GCN (nn_GCNModel), SPMD across 8 NeuronCores.

Strategy (graph/data parallel per the sharding hint):
  - Nodes are partitioned across the 8 cores (6250 real rows/core, padded to
    6272 = 49 blocks of 128).  Each core owns the edges whose DESTINATION
    falls in its shard, pre-sorted by destination block on the host and
    padded so every core sees the same per-block chunk structure (SPMD: one
    NEFF, 8 cores).
  - GCN normalization (deg -> dis -> per-edge norm = dis[row]*w*dis[col]) is
    computed on the HOST; the kernel receives per-edge norms (bf16) and the
    per-node self-loop coefficients 1/deg directly.
  - Per layer: each core computes h = X_shard @ W for its rows (bf16), the
    padded h table is AllGather'd to every core's DRAM, then each core
    gathers the source rows of its edges with `dma_gather` (one gather per
    (dest-block, lo/hi table half); int16 indices force the lo/hi split at
    32768).  Each gather's index stream is padded to the max real count over
    cores with index 0 and then with trailing -1s, and num_idxs_reg is set
    to that count, so the SWDGE generator skips the chunk-rounding padding
    entirely (the matmul masks stale lanes via norm=0).
  - Norm-scaled one-hot selection matrices for a whole 4-block span are
    built with two broadcast tensor_tensor ops on the Vector engine
    (M[e,c,d] = (iota[d]==rl[e,c]) * norm[e,c], bf16), and aggregation runs
    as TensorE matmul accumulation in PSUM: agg^T[f,d] += G[e,f]^T @ M[e,d].
  - Self-loop edges never enter the gather stream: their contribution
    h[d]*1/deg[d] is added per dest block as one extra accumulation matmul
    lhsT=h_block (node-major, local) rhs=diag((iota==lane)*dinv).
  - agg^T comes out feature-major [F, nodes] — the stationary operand layout
    the next layer's matmul wants; bias+ReLU ride the PSUM->SBUF copy on the
    Scalar engine.
  - All per-core constants ship as ONE packed f32 blob (single DMA/sem).

Self-contained: hardcodes all shapes; host side only computes norms,
reorders/pads indices and concatenates shard outputs.
"""

import os
import sys

import numpy as np
import ml_dtypes

for _p in ("/opt/trn_rl_repo", "/root/.axon_site/_ro/trn_rl_repo"):
    if os.path.isdir(_p) and _p not in sys.path:
        sys.path.append(_p)

from concourse import bacc, mybir, tile  # noqa: E402
from concourse.bass_utils import run_bass_kernel_spmd  # noqa: E402

F32 = mybir.dt.float32
BF16 = mybir.dt.bfloat16
I16 = mybir.dt.int16
BF = ml_dtypes.bfloat16

N_NODES = 50000
NCORES = 8
S_REAL = N_NODES // NCORES           # 6250 real nodes per core
NB = (S_REAL + 127) // 128           # 49 dest blocks per core
S = NB * 128                         # 6272 padded shard size
NPAD = NCORES * S                    # 50176 padded global nodes
SPLIT = 32768                        # int16 index limit for dma_gather
SPAN = 4                             # dest blocks per span (M build / g tile)
WD = 128                             # h-table row width (bf16 -> 256B rows)
MAX_IDX_PER_GATHER = 8192
# (F_in, F_out) per layer
LAYERS = [(128, 100), (100, 100), (100, 50), (50, 6)]

TRACE = False
TRACE_KW = {}
LAST_RESULTS = None


def _wrap_idx16(flat):
    """dma_gather index layout: idx i at (partition i%16, column i//16),
    replicated to all 8 16-partition groups."""
    n = len(flat)
    assert n % 16 == 0
    w = flat.reshape(n // 16, 16).T.astype(np.int16)     # [16, n/16]
    return np.tile(w, (8, 1))                            # [128, n/16]


def _preprocess(edge_index, edge_weight):
    """Shard + sort non-self edges by (dest block, lo/hi col half).

    Returns (spans, block_cols, CH_TOT, idx16, rl_a, nrm_a, dinv_a):
      spans: list of (j0, blocks, groups); groups = (b, h, jc, nch, maxcnt)
      block_cols: per block, the ordered chunk-column indices
    """
    row = edge_index[0].astype(np.int64)
    col = edge_index[1].astype(np.int64)
    w = edge_weight.astype(np.float32)

    # GCN renormalization coefficients on host (self-loop weight 1 included
    # in deg, but self-loop edges are applied separately in the kernel).
    deg = np.ones(N_NODES, np.float32)
    np.add.at(deg, row, w)
    dis = (1.0 / np.sqrt(deg)).astype(np.float32)
    norm = dis[row] * w * dis[col]
    dinv_full = (1.0 / deg).astype(np.float32)

    core = row // S_REAL
    r_in = row % S_REAL
    blk = r_in // 128
    rloc = (r_in % 128).astype(np.float32)
    colp = ((col // S_REAL) * S + col % S_REAL).astype(np.int64)
    hi = (colp >= SPLIT).astype(np.int64)

    counts = np.zeros((NCORES, NB, 2), np.int64)
    np.add.at(counts, (core, blk, hi), 1)
    maxcnt = counts.max(axis=0)                          # [NB, 2]
    CHP = -(-maxcnt // 128)                              # [NB, 2] chunks
    CH_TOT = int(CHP.sum())

    # span/chunk-column layout
    spans = []
    block_cols = [[] for _ in range(NB)]
    part_col = {}
    j = 0
    for s0 in range(0, NB, SPAN):
        blocks = list(range(s0, min(s0 + SPAN, NB)))
        j0 = j
        n_lo = n_hi = 0
        for h in range(2):
            for b in blocks:
                nch = int(CHP[b, h])
                part_col[(b, h)] = j
                block_cols[b].extend(range(j, j + nch))
                j += nch
                if h == 0:
                    n_lo += nch
                else:
                    n_hi += nch
        assert n_lo * 128 <= MAX_IDX_PER_GATHER
        assert n_hi * 128 <= MAX_IDX_PER_GATHER
        spans.append((j0, n_lo, n_hi, blocks))
    assert j == CH_TOT

    idx16 = np.zeros((NCORES, 128, CH_TOT * 8), np.int16)
    rl_a = np.zeros((NCORES, 128, CH_TOT), np.float32)
    nrm_a = np.zeros((NCORES, 128, CH_TOT), np.float32)

    order = np.lexsort((hi, blk, core))
    scolp, srloc, snrm = colp[order], rloc[order], norm[order]
    bounds = np.searchsorted(
        core[order] * (NB * 2) + blk[order] * 2 + hi[order],
        np.arange(NCORES * NB * 2 + 1))

    for c in range(NCORES):
        for b in range(NB):
            for h in range(2):
                nch = int(CHP[b, h])
                if nch == 0:
                    continue
                jc = part_col[(b, h)]
                k = (c * NB + b) * 2 + h
                s0, s1 = bounds[k], bounds[k + 1]
                cnt = s1 - s0
                n = nch * 128
                ii = np.zeros(n, np.int64)
                ii[:cnt] = scolp[s0:s1] - h * SPLIT
                rr = np.zeros(n, np.float32)
                rr[:cnt] = srloc[s0:s1]
                ww = np.zeros(n, np.float32)
                ww[:cnt] = snrm[s0:s1]
                rl_a[c, :, jc:jc + nch] = rr.reshape(nch, 128).T
                nrm_a[c, :, jc:jc + nch] = ww.reshape(nch, 128).T
                idx16[c, :, jc * 8:jc * 8 + n // 16] = _wrap_idx16(ii)

    dinv_a = np.zeros((NCORES, 128, NB), np.float32)
    for c in range(NCORES):
        loc = np.arange(S)
        n_glob = c * S_REAL + loc
        valid = loc < S_REAL
        da = np.where(valid, dinv_full[np.minimum(n_glob, N_NODES - 1)], 0.0)
        dinv_a[c] = da.reshape(NB, 128).T
    return spans, block_cols, CH_TOT, idx16, rl_a, nrm_a, dinv_a


def _blob_offsets(CH_TOT):
    """Column layout of the packed per-core constant blob [128, NCOLS] f32."""
    CH2 = (CH_TOT + 1) // 2 * 2
    off = {}
    o = 0
    NB2 = (NB + 1) // 2 * 2
    for k, n in (("idx16", CH_TOT * 4), ("rl16", CH2 // 2),
                 ("nrm16", CH2 // 2), ("iota", 64), ("xT", S // 2),
                 ("dinv", NB2 // 2), ("ic", 1)):
        off[k] = o
        o += n
    for l, (fin, fout) in enumerate(LAYERS):
        off[f"W{l}"] = o
        o += fout // 2
    for l, (fin, fout) in enumerate(LAYERS):
        off[f"b{l}"] = o
        o += 1
    return off, o


def _build(spans, block_cols, CH_TOT):
    nc = bacc.Bacc(None, num_devices=NCORES)
    AGRP = [list(range(NCORES))]
    AF = mybir.ActivationFunctionType
    OP = mybir.AluOpType

    off, ncols = _blob_offsets(CH_TOT)
    blob_p = nc.declare_dram_parameter("blob", [128, ncols], F32, isOutput=False)
    out_p = nc.declare_dram_parameter("out", [LAYERS[-1][1], S], F32, isOutput=True)

    with tile.TileContext(nc, num_cores=NCORES) as tc:
        with (
            tc.tile_pool(name="const", bufs=1) as cpool,
            tc.tile_pool(name="xpool", bufs=2) as xpool,
            tc.tile_pool(name="dram", bufs=1, space="DRAM") as dpool,
            tc.tile_pool(name="psum", bufs=2, space="PSUM") as ppool,
            tc.tile_pool(name="work", bufs=3) as wpool,
        ):
            blob = cpool.tile([128, ncols], F32)
            nc.sync.dma_start(out=blob[:], in_=blob_p[:])

            idx16_sb = blob[:, off["idx16"]:off["idx16"] + CH_TOT * 4].bitcast(I16)
            rl16 = blob[:, off["rl16"]:off["nrm16"]].bitcast(BF16)
            nrm16 = blob[:, off["nrm16"]:off["iota"]].bitcast(BF16)
            iota_b = blob[:, off["iota"]:off["iota"] + 64].bitcast(BF16)
            xT0 = blob[:, off["xT"]:off["xT"] + S // 2].bitcast(BF16)
            dinv_sb = blob[:, off["dinv"]:off["ic"]].bitcast(BF16)
            ic_sb = blob[:, off["ic"]:off["ic"] + 1]
            W_sb = [blob[:LAYERS[l][0],
                         off[f"W{l}"]:off[f"W{l}"] + LAYERS[l][1] // 2].bitcast(BF16)
                    for l in range(4)]
            b_sb = [blob[:LAYERS[l][1], off[f"b{l}"]:off[f"b{l}"] + 1]
                    for l in range(4)]

            # identity and per-block self-loop diagonals (layer-independent)
            ident = cpool.tile([128, 128], BF16)
            nc.vector.tensor_scalar(ident[:], iota_b, ic_sb[:, 0:1], None,
                                    OP.is_equal)
            diagall = cpool.tile([128, NB * 128], BF16)
            nc.vector.tensor_tensor(
                out=diagall[:].rearrange("p (b d) -> p b d", d=128),
                in0=ident[:].unsqueeze(1).broadcast_to((128, NB, 128)),
                in1=dinv_sb[:, 0:NB].unsqueeze(2).broadcast_to((128, NB, 128)),
                op=OP.mult)

            # ---- layers ----
            xT = xT0
            for l, (fin, fout) in enumerate(LAYERS):
                # h = X_shard @ W (node-major bf16, padded to WD), one DMA out
                h_all = wpool.tile([128, NB * WD], BF16, tag="hall", bufs=1)
                for b in range(NB):
                    ps_h = ppool.tile([128, fout], F32, tag="ps_small")
                    nc.tensor.matmul(
                        out=ps_h[:], lhsT=xT[:fin, b * 128:(b + 1) * 128],
                        rhs=W_sb[l], start=True, stop=True)
                    nc.scalar.activation(
                        h_all[:, b * WD:b * WD + fout], ps_h[:], AF.Copy)
                hsh = dpool.tile([S, WD], BF16, name=f"hsh{l}")
                nc.sync.dma_start(
                    out=hsh.rearrange("(b p) w -> p b w", p=128),
                    in_=h_all[:].rearrange("p (b w) -> p b w", w=WD))

                htab = dpool.tile([NPAD, WD], BF16, addr_space="Shared",
                                  name=f"htab{l}")
                nc.gpsimd.collective_compute(
                    "AllGather", OP.bypass, replica_groups=AGRP,
                    ins=[hsh[:]], outs=[htab[:]])

                # aggregation: agg^T[f, d] += G[e, f]^T @ M[e, d]
                xT_next = xpool.tile([fout, S], BF16 if l + 1 < len(LAYERS)
                                     else F32, tag="xT", name=f"xT{l + 1}_sb")
                for (j0, n_lo, n_hi, blocks) in spans:
                    nch_s = n_lo + n_hi
                    g = wpool.tile([128, nch_s * WD], BF16, tag="g", bufs=3)
                    for h, tab, o0, nch in ((0, htab[0:SPLIT, :], 0, n_lo),
                                            (1, htab[SPLIT:NPAD, :],
                                             n_lo * WD, n_hi)):
                        if nch == 0:
                            continue
                        i16 = (j0 + (0 if h == 0 else n_lo)) * 8
                        nc.gpsimd.dma_gather(
                            g[:, o0:o0 + nch * WD].rearrange(
                                "p (c w) -> p c w", w=WD),
                            tab, idx16_sb[:, i16:i16 + nch * 8],
                            nch * 128, nch * 128, WD, single_packet=False)
                    # norm-scaled one-hots for the whole span (2 DVE ops)
                    mspan = wpool.tile([128, nch_s * 128], BF16, tag="m", bufs=2)
                    m3 = mspan[:].rearrange("p (c w) -> p c w", w=128)
                    nc.vector.tensor_tensor(
                        out=m3,
                        in0=iota_b.unsqueeze(1).broadcast_to((128, nch_s, 128)),
                        in1=rl16[:, j0:j0 + nch_s].unsqueeze(2).broadcast_to(
                            (128, nch_s, 128)),
                        op=OP.is_equal)
                    nc.vector.tensor_tensor(
                        out=m3, in0=m3,
                        in1=nrm16[:, j0:j0 + nch_s].unsqueeze(2).broadcast_to(
                            (128, nch_s, 128)),
                        op=OP.mult)
                    for b in blocks:
                        cols = block_cols[b]
                        ps_a = ppool.tile([fout, 128], F32, tag="psa", bufs=4)
                        for k, j in enumerate(cols):
                            o0 = (j - j0) * WD
                            nc.tensor.matmul(
                                out=ps_a[:], lhsT=g[:, o0:o0 + fout],
                                rhs=mspan[:, (j - j0) * 128:(j - j0 + 1) * 128],
                                start=(k == 0), stop=False)
                        # self-loop: agg^T[:, d] += h[d, :]^T * dinv[d]
                        nc.tensor.matmul(
                            out=ps_a[:], lhsT=h_all[:, b * WD:b * WD + fout],
                            rhs=diagall[:, b * 128:(b + 1) * 128],
                            start=(len(cols) == 0), stop=True)
                        # epilogue: +bias, ReLU on the PSUM->SBUF copy
                        nc.scalar.activation(
                            xT_next[:, b * 128:(b + 1) * 128], ps_a[:],
                            AF.Relu if l + 1 < len(LAYERS) else AF.Identity,
                            bias=b_sb[l][:, 0:1])
                xT = xT_next

            nc.sync.dma_start(out=out_p[:], in_=xT[:])
    nc.finalize()   # Bacc: reg alloc + event-sem wait splitting
    return nc


def kernel(x, edge_index, edge_weight, W0, b0, W1, b1, W2, b2, W3, b3):
    global LAST_RESULTS
    x = np.ascontiguousarray(np.asarray(x, np.float32))
    spans, block_cols, CH_TOT, idx16, rl_a, nrm_a, dinv_a = _preprocess(
        np.asarray(edge_index), np.asarray(edge_weight))

    nc = _build(spans, block_cols, CH_TOT)

    off, ncols = _blob_offsets(CH_TOT)
    CH2 = (CH_TOT + 1) // 2 * 2
    Ws = [np.asarray(W, np.float32).astype(BF) for W in (W0, W1, W2, W3)]
    bs = [np.asarray(b, np.float32).reshape(-1) for b in (b0, b1, b2, b3)]

    in_maps = []
    for c in range(NCORES):
        blob = np.zeros((128, ncols), np.float32)
        blob[:, off["idx16"]:off["idx16"] + CH_TOT * 4] = idx16[c].view(np.float32)
        rl16 = np.zeros((128, CH2), BF)
        rl16[:, :CH_TOT] = rl_a[c].astype(BF)
        blob[:, off["rl16"]:off["nrm16"]] = rl16.view(np.float32)
        nrm16 = np.zeros((128, CH2), BF)
        nrm16[:, :CH_TOT] = nrm_a[c].astype(BF)
        blob[:, off["nrm16"]:off["iota"]] = nrm16.view(np.float32)
        blob[:, off["iota"]:off["iota"] + 64] = np.broadcast_to(
            np.arange(128, dtype=np.float32).astype(BF).view(np.float32)[None, :],
            (128, 64))
        xb = np.zeros((128, S), BF)
        xb[:, :S_REAL] = x[c * S_REAL:(c + 1) * S_REAL].T.astype(BF)
        blob[:, off["xT"]:off["xT"] + S // 2] = xb.view(np.float32)
        NB2 = (NB + 1) // 2 * 2
        dv16 = np.zeros((128, NB2), BF)
        dv16[:, :NB] = dinv_a[c].astype(BF)
        blob[:, off["dinv"]:off["ic"]] = dv16.view(np.float32)
        blob[:, off["ic"]] = np.arange(128, dtype=np.float32)
        for l, (fin, fout) in enumerate(LAYERS):
            blob[:fin, off[f"W{l}"]:off[f"W{l}"] + fout // 2] = Ws[l].view(np.float32)
            blob[:fout, off[f"b{l}"]] = bs[l]
        in_maps.append({"blob": blob})

    res = run_bass_kernel_spmd(nc, in_maps, core_ids=list(range(NCORES)),
                               trace=TRACE, trace_kwargs=dict(TRACE_KW))
    LAST_RESULTS = res
    out = np.concatenate(
        [res.results[c]["out"][:, :S_REAL].T for c in range(NCORES)], axis=0)
    return np.ascontiguousarray(out.astype(np.float32))


# revision 6
# speedup vs baseline: 1.2363x; 1.0079x over previous
"""Trainium2 Bass kernel for a 4-layer GCN (nn_GCNModel), SPMD across 8 NeuronCores.

Strategy (graph/data parallel per the sharding hint):
  - Nodes are partitioned across the 8 cores (6250 real rows/core, padded to
    6272 = 49 blocks of 128).  Each core owns the edges whose DESTINATION
    falls in its shard, pre-sorted by destination block on the host and
    padded so every core sees the same per-block chunk structure (SPMD: one
    NEFF, 8 cores).
  - GCN normalization (deg -> dis -> per-edge norm = dis[row]*w*dis[col]) is
    computed on the HOST; the kernel receives per-edge norms (bf16) and the
    per-node self-loop coefficients 1/deg directly.
  - Per layer: each core computes h = X_shard @ W for its rows (bf16), the
    padded h table is AllGather'd to every core's DRAM, then each core
    gathers the source rows of its edges with `dma_gather` (one gather per
    (dest-block, lo/hi table half); int16 indices force the lo/hi split at
    32768).  Each gather's index stream is padded to the max real count over
    cores with index 0 and then with trailing -1s, and num_idxs_reg is set
    to that count, so the SWDGE generator skips the chunk-rounding padding
    entirely (the matmul masks stale lanes via norm=0).
  - Norm-scaled one-hot selection matrices for a whole 4-block span are
    built with two broadcast tensor_tensor ops on the Vector engine
    (M[e,c,d] = (iota[d]==rl[e,c]) * norm[e,c], bf16), and aggregation runs
    as TensorE matmul accumulation in PSUM: agg^T[f,d] += G[e,f]^T @ M[e,d].
  - Self-loop edges never enter the gather stream: their contribution
    h[d]*1/deg[d] is added per dest block as one extra accumulation matmul
    lhsT=h_block (node-major, local) rhs=diag((iota==lane)*dinv).
  - agg^T comes out feature-major [F, nodes] — the stationary operand layout
    the next layer's matmul wants; bias+ReLU ride the PSUM->SBUF copy on the
    Scalar engine.
  - All per-core constants ship as ONE packed f32 blob (single DMA/sem).

Self-contained: hardcodes all shapes; host side only computes norms,
reorders/pads indices and concatenates shard outputs.
"""

import os
import sys

import numpy as np
import ml_dtypes

for _p in ("/opt/trn_rl_repo", "/root/.axon_site/_ro/trn_rl_repo"):
    if os.path.isdir(_p) and _p not in sys.path:
        sys.path.append(_p)

from concourse import bacc, mybir, tile  # noqa: E402
from concourse.bass_utils import run_bass_kernel_spmd  # noqa: E402

F32 = mybir.dt.float32
BF16 = mybir.dt.bfloat16
I16 = mybir.dt.int16
BF = ml_dtypes.bfloat16

N_NODES = 50000
NCORES = 8
S_REAL = N_NODES // NCORES           # 6250 real nodes per core
NB = (S_REAL + 127) // 128           # 49 dest blocks per core
S = NB * 128                         # 6272 padded shard size
NPAD = NCORES * S                    # 50176 padded global nodes
SPLIT = 32768                        # int16 index limit for dma_gather
SPAN = 4                             # dest blocks per span (M build / g tile)
WD = 128                             # h-table row width (bf16 -> 256B rows)
MAX_IDX_PER_GATHER = 8192
# (F_in, F_out) per layer
LAYERS = [(128, 100), (100, 100), (100, 50), (50, 6)]

TRACE = False
TRACE_KW = {}
LAST_RESULTS = None


def _wrap_idx16(flat):
    """dma_gather index layout: idx i at (partition i%16, column i//16),
    replicated to all 8 16-partition groups."""
    n = len(flat)
    assert n % 16 == 0
    w = flat.reshape(n // 16, 16).T.astype(np.int16)     # [16, n/16]
    return np.tile(w, (8, 1))                            # [128, n/16]


def _preprocess(edge_index, edge_weight):
    """Shard + sort non-self edges by (dest block, lo/hi col half).

    Returns (spans, block_cols, CH_TOT, idx16, rl_a, nrm_a, dinv_a):
      spans: list of (j0, blocks, groups); groups = (b, h, jc, nch, maxcnt)
      block_cols: per block, the ordered chunk-column indices
    """
    row = edge_index[0].astype(np.int64)
    col = edge_index[1].astype(np.int64)
    w = edge_weight.astype(np.float32)

    # GCN renormalization coefficients on host (self-loop weight 1 included
    # in deg, but self-loop edges are applied separately in the kernel).
    deg = np.ones(N_NODES, np.float32)
    np.add.at(deg, row, w)
    dis = (1.0 / np.sqrt(deg)).astype(np.float32)
    norm = dis[row] * w * dis[col]
    dinv_full = (1.0 / deg).astype(np.float32)

    core = row // S_REAL
    r_in = row % S_REAL
    blk = r_in // 128
    rloc = (r_in % 128).astype(np.float32)
    colp = ((col // S_REAL) * S + col % S_REAL).astype(np.int64)
    hi = (colp >= SPLIT).astype(np.int64)

    counts = np.zeros((NCORES, NB, 2), np.int64)
    np.add.at(counts, (core, blk, hi), 1)
    maxcnt = counts.max(axis=0)                          # [NB, 2]
    CHP = -(-maxcnt // 128)                              # [NB, 2] chunks
    CH_TOT = int(CHP.sum())

    # span/chunk-column layout
    spans = []
    block_cols = [[] for _ in range(NB)]
    part_col = {}
    j = 0
    for s0 in range(0, NB, SPAN):
        blocks = list(range(s0, min(s0 + SPAN, NB)))
        j0 = j
        n_lo = n_hi = 0
        for h in range(2):
            for b in blocks:
                nch = int(CHP[b, h])
                part_col[(b, h)] = j
                block_cols[b].extend(range(j, j + nch))
                j += nch
                if h == 0:
                    n_lo += nch
                else:
                    n_hi += nch
        assert n_lo * 128 <= MAX_IDX_PER_GATHER
        assert n_hi * 128 <= MAX_IDX_PER_GATHER
        spans.append((j0, n_lo, n_hi, blocks))
    assert j == CH_TOT

    idx16 = np.zeros((NCORES, 128, CH_TOT * 8), np.int16)
    rl_a = np.zeros((NCORES, 128, CH_TOT), np.float32)
    nrm_a = np.zeros((NCORES, 128, CH_TOT), np.float32)

    order = np.lexsort((hi, blk, core))
    scolp, srloc, snrm = colp[order], rloc[order], norm[order]
    bounds = np.searchsorted(
        core[order] * (NB * 2) + blk[order] * 2 + hi[order],
        np.arange(NCORES * NB * 2 + 1))

    for c in range(NCORES):
        for b in range(NB):
            for h in range(2):
                nch = int(CHP[b, h])
                if nch == 0:
                    continue
                jc = part_col[(b, h)]
                k = (c * NB + b) * 2 + h
                s0, s1 = bounds[k], bounds[k + 1]
                cnt = s1 - s0
                n = nch * 128
                ii = np.zeros(n, np.int64)
                ii[:cnt] = scolp[s0:s1] - h * SPLIT
                rr = np.zeros(n, np.float32)
                rr[:cnt] = srloc[s0:s1]
                ww = np.zeros(n, np.float32)
                ww[:cnt] = snrm[s0:s1]
                rl_a[c, :, jc:jc + nch] = rr.reshape(nch, 128).T
                nrm_a[c, :, jc:jc + nch] = ww.reshape(nch, 128).T
                idx16[c, :, jc * 8:jc * 8 + n // 16] = _wrap_idx16(ii)

    dinv_a = np.zeros((NCORES, 128, NB), np.float32)
    for c in range(NCORES):
        loc = np.arange(S)
        n_glob = c * S_REAL + loc
        valid = loc < S_REAL
        da = np.where(valid, dinv_full[np.minimum(n_glob, N_NODES - 1)], 0.0)
        dinv_a[c] = da.reshape(NB, 128).T
    return spans, block_cols, CH_TOT, idx16, rl_a, nrm_a, dinv_a


def _blob_offsets(CH_TOT):
    """Column layout of the packed per-core constant blob [128, NCOLS] f32."""
    CH2 = (CH_TOT + 1) // 2 * 2
    off = {}
    o = 0
    NB2 = (NB + 1) // 2 * 2
    for k, n in (("idx16", CH_TOT * 4), ("rl16", CH2 // 2),
                 ("nrm16", CH2 // 2), ("iota", 64), ("xT", S // 2),
                 ("dinv", NB2 // 2), ("ic", 1)):
        off[k] = o
        o += n
    for l, (fin, fout) in enumerate(LAYERS):
        off[f"W{l}"] = o
        o += fout // 2
    for l, (fin, fout) in enumerate(LAYERS):
        off[f"b{l}"] = o
        o += 1
    return off, o


def _build(spans, block_cols, CH_TOT):
    nc = bacc.Bacc(None, num_devices=NCORES)
    AGRP = [list(range(NCORES))]
    AF = mybir.ActivationFunctionType
    OP = mybir.AluOpType

    off, ncols = _blob_offsets(CH_TOT)
    blob_p = nc.declare_dram_parameter("blob", [128, ncols], F32, isOutput=False)
    out_p = nc.declare_dram_parameter("out", [LAYERS[-1][1], S], F32, isOutput=True)

    with tile.TileContext(nc, num_cores=NCORES) as tc:
        with (
            tc.tile_pool(name="const", bufs=1) as cpool,
            tc.tile_pool(name="xpool", bufs=2) as xpool,
            tc.tile_pool(name="dram", bufs=1, space="DRAM") as dpool,
            tc.tile_pool(name="psum", bufs=2, space="PSUM") as ppool,
            tc.tile_pool(name="work", bufs=3) as wpool,
        ):
            blob = cpool.tile([128, ncols], F32)
            nc.sync.dma_start(out=blob[:], in_=blob_p[:])

            idx16_sb = blob[:, off["idx16"]:off["idx16"] + CH_TOT * 4].bitcast(I16)
            rl16 = blob[:, off["rl16"]:off["nrm16"]].bitcast(BF16)
            nrm16 = blob[:, off["nrm16"]:off["iota"]].bitcast(BF16)
            iota_b = blob[:, off["iota"]:off["iota"] + 64].bitcast(BF16)
            xT0 = blob[:, off["xT"]:off["xT"] + S // 2].bitcast(BF16)
            dinv_sb = blob[:, off["dinv"]:off["ic"]].bitcast(BF16)
            ic_sb = blob[:, off["ic"]:off["ic"] + 1]
            W_sb = [blob[:LAYERS[l][0],
                         off[f"W{l}"]:off[f"W{l}"] + LAYERS[l][1] // 2].bitcast(BF16)
                    for l in range(4)]
            b_sb = [blob[:LAYERS[l][1], off[f"b{l}"]:off[f"b{l}"] + 1]
                    for l in range(4)]

            # identity and per-block self-loop diagonals (layer-independent)
            ident = cpool.tile([128, 128], BF16)
            nc.vector.tensor_scalar(ident[:], iota_b, ic_sb[:, 0:1], None,
                                    OP.is_equal)
            diagall = cpool.tile([128, NB * 128], BF16)
            nc.vector.tensor_tensor(
                out=diagall[:].rearrange("p (b d) -> p b d", d=128),
                in0=ident[:].unsqueeze(1).broadcast_to((128, NB, 128)),
                in1=dinv_sb[:, 0:NB].unsqueeze(2).broadcast_to((128, NB, 128)),
                op=OP.mult)

            # ---- layers ----
            # layer-0 h = X_shard @ W0 up front; later layers' h matmuls are
            # interleaved into the previous layer's aggregation epilogue.
            xT = xT0
            h_all = wpool.tile([128, NB * WD], BF16, tag="hall", bufs=2)
            for b in range(NB):
                ps_h = ppool.tile([128, LAYERS[0][1]], F32, tag="ps_small")
                nc.tensor.matmul(
                    out=ps_h[:], lhsT=xT0[:LAYERS[0][0], b * 128:(b + 1) * 128],
                    rhs=W_sb[0], start=True, stop=True)
                nc.scalar.activation(
                    h_all[:, b * WD:b * WD + LAYERS[0][1]], ps_h[:], AF.Copy)
            for l, (fin, fout) in enumerate(LAYERS):
                hsh = dpool.tile([S, WD], BF16, name=f"hsh{l}")
                nc.sync.dma_start(
                    out=hsh.rearrange("(b p) w -> p b w", p=128),
                    in_=h_all[:].rearrange("p (b w) -> p b w", w=WD))

                htab = dpool.tile([NPAD, WD], BF16, addr_space="Shared",
                                  name=f"htab{l}")
                nc.gpsimd.collective_compute(
                    "AllGather", OP.bypass, replica_groups=AGRP,
                    ins=[hsh[:]], outs=[htab[:]])

                # aggregation: agg^T[f, d] += G[e, f]^T @ M[e, d]
                xT_next = xpool.tile([fout, S], BF16 if l + 1 < len(LAYERS)
                                     else F32, tag="xT", name=f"xT{l + 1}_sb")
                if l + 1 < len(LAYERS):
                    h_next = wpool.tile([128, NB * WD], BF16, tag="hall",
                                        bufs=2)
                    fo2 = LAYERS[l + 1][1]
                for (j0, n_lo, n_hi, blocks) in spans:
                    nch_s = n_lo + n_hi
                    g = wpool.tile([128, nch_s * WD], BF16, tag="g", bufs=3)
                    for h, tab, o0, nch in ((0, htab[0:SPLIT, :], 0, n_lo),
                                            (1, htab[SPLIT:NPAD, :],
                                             n_lo * WD, n_hi)):
                        if nch == 0:
                            continue
                        i16 = (j0 + (0 if h == 0 else n_lo)) * 8
                        nc.gpsimd.dma_gather(
                            g[:, o0:o0 + nch * WD].rearrange(
                                "p (c w) -> p c w", w=WD),
                            tab, idx16_sb[:, i16:i16 + nch * 8],
                            nch * 128, nch * 128, WD, single_packet=False)
                    # norm-scaled one-hots for the whole span (2 DVE ops)
                    mspan = wpool.tile([128, nch_s * 128], BF16, tag="m", bufs=2)
                    m3 = mspan[:].rearrange("p (c w) -> p c w", w=128)
                    nc.vector.tensor_tensor(
                        out=m3,
                        in0=iota_b.unsqueeze(1).broadcast_to((128, nch_s, 128)),
                        in1=rl16[:, j0:j0 + nch_s].unsqueeze(2).broadcast_to(
                            (128, nch_s, 128)),
                        op=OP.is_equal)
                    nc.vector.tensor_tensor(
                        out=m3, in0=m3,
                        in1=nrm16[:, j0:j0 + nch_s].unsqueeze(2).broadcast_to(
                            (128, nch_s, 128)),
                        op=OP.mult)
                    for b in blocks:
                        cols = block_cols[b]
                        ps_a = ppool.tile([fout, 128], F32, tag="psa", bufs=4)
                        for k, j in enumerate(cols):
                            o0 = (j - j0) * WD
                            nc.tensor.matmul(
                                out=ps_a[:], lhsT=g[:, o0:o0 + fout],
                                rhs=mspan[:, (j - j0) * 128:(j - j0 + 1) * 128],
                                start=(k == 0), stop=False)
                        # self-loop: agg^T[:, d] += h[d, :]^T * dinv[d]
                        nc.tensor.matmul(
                            out=ps_a[:], lhsT=h_all[:, b * WD:b * WD + fout],
                            rhs=diagall[:, b * 128:(b + 1) * 128],
                            start=(len(cols) == 0), stop=True)
                        # epilogue: +bias, ReLU on the PSUM->SBUF copy
                        nc.scalar.activation(
                            xT_next[:, b * 128:(b + 1) * 128], ps_a[:],
                            AF.Relu if l + 1 < len(LAYERS) else AF.Identity,
                            bias=b_sb[l][:, 0:1])
                        # next layer's h for this block rides the slack
                        if l + 1 < len(LAYERS):
                            ps_h = ppool.tile([128, fo2], F32, tag="ps_small")
                            nc.tensor.matmul(
                                out=ps_h[:],
                                lhsT=xT_next[:fout, b * 128:(b + 1) * 128],
                                rhs=W_sb[l + 1], start=True, stop=True)
                            nc.scalar.activation(
                                h_next[:, b * WD:b * WD + fo2], ps_h[:],
                                AF.Copy)
                if l + 1 < len(LAYERS):
                    h_all = h_next
                xT = xT_next

            nc.sync.dma_start(out=out_p[:], in_=xT[:])
    nc.finalize()   # Bacc: reg alloc + event-sem wait splitting
    return nc


def kernel(x, edge_index, edge_weight, W0, b0, W1, b1, W2, b2, W3, b3):
    global LAST_RESULTS
    x = np.ascontiguousarray(np.asarray(x, np.float32))
    spans, block_cols, CH_TOT, idx16, rl_a, nrm_a, dinv_a = _preprocess(
        np.asarray(edge_index), np.asarray(edge_weight))

    nc = _build(spans, block_cols, CH_TOT)

    off, ncols = _blob_offsets(CH_TOT)
    CH2 = (CH_TOT + 1) // 2 * 2
    Ws = [np.asarray(W, np.float32).astype(BF) for W in (W0, W1, W2, W3)]
    bs = [np.asarray(b, np.float32).reshape(-1) for b in (b0, b1, b2, b3)]

    in_maps = []
    for c in range(NCORES):
        blob = np.zeros((128, ncols), np.float32)
        blob[:, off["idx16"]:off["idx16"] + CH_TOT * 4] = idx16[c].view(np.float32)
        rl16 = np.zeros((128, CH2), BF)
        rl16[:, :CH_TOT] = rl_a[c].astype(BF)
        blob[:, off["rl16"]:off["nrm16"]] = rl16.view(np.float32)
        nrm16 = np.zeros((128, CH2), BF)
        nrm16[:, :CH_TOT] = nrm_a[c].astype(BF)
        blob[:, off["nrm16"]:off["iota"]] = nrm16.view(np.float32)
        blob[:, off["iota"]:off["iota"] + 64] = np.broadcast_to(
            np.arange(128, dtype=np.float32).astype(BF).view(np.float32)[None, :],
            (128, 64))
        xb = np.zeros((128, S), BF)
        xb[:, :S_REAL] = x[c * S_REAL:(c + 1) * S_REAL].T.astype(BF)
        blob[:, off["xT"]:off["xT"] + S // 2] = xb.view(np.float32)
        NB2 = (NB + 1) // 2 * 2
        dv16 = np.zeros((128, NB2), BF)
        dv16[:, :NB] = dinv_a[c].astype(BF)
        blob[:, off["dinv"]:off["ic"]] = dv16.view(np.float32)
        blob[:, off["ic"]] = np.arange(128, dtype=np.float32)
        for l, (fin, fout) in enumerate(LAYERS):
            blob[:fin, off[f"W{l}"]:off[f"W{l}"] + fout // 2] = Ws[l].view(np.float32)
            blob[:fout, off[f"b{l}"]] = bs[l]
        in_maps.append({"blob": blob})

    res = run_bass_kernel_spmd(nc, in_maps, core_ids=list(range(NCORES)),
                               trace=TRACE, trace_kwargs=dict(TRACE_KW))
    LAST_RESULTS = res
    out = np.concatenate(
        [res.results[c]["out"][:, :S_REAL].T for c in range(NCORES)], axis=0)
    return np.ascontiguousarray(out.astype(np.float32))
